# revision 8
# baseline (speedup 1.0000x reference)
"""SPP (spatial pyramid pooling) kernel for Trainium2, 8 NeuronCores.

Input  x  : [16, 256, 64, 64] f32
Output    : [16, 5376, 13, 13] f32

Math: windows are 16x16 at stride 4 -> 13x13 window grid. Levels use
sub-cells of 16/8/4 pixels, all aligned to multiples of 4, so everything
reduces to the non-overlapping 4x4 block-max P2 [16,16] per (b,c) image:
  lvl2 plane (q,r) = P2[q+i, r+j]              (16 planes of 13x13)
  P1 = 2x2 stride-1 max of P2 -> [15,15];  lvl1 plane (q,r) = P1[2q+i, 2r+j]
  P0 = 4x4 stride-1 max of P2 -> [13,13];  lvl0 plane    = P0
Output channel order: [lvl0: c][lvl1: c*4+q*2+r][lvl2: c*16+q*4+r].

Precision: bf16 after the first row-max (max commutes exactly with
monotone round-to-nearest, so every output equals the bf16 rounding of
the exact f32 max: rel err <= 2^-9, vs the 2e-2 harness gate).  Output
DRAM tensor is bf16 (halves store bytes); host upcasts to f32.

Schedule: ALL DMA (loads and stores) goes on the single SP HWDGE ring,
which drains strictly FIFO.  The 8 half-tile loads are enqueued
up-front, so the 8.39 MB load stream runs at full rate (~430 GB/s,
done ~28 us) with zero store competition; the 1.82 MB of bf16 stores
sit behind them in the ring (enqueued as their copies finish) and
drain during the last tile's gather-copy tail, ending right at the
byte floor.  Engine balance: VectorE = max trees, ScalarE = lvl2
gather copies, GpSimd = lvl1 gather copies (compute COPY, no SWDGE so
no Q7 drain at kernel end), Sync/SP = every dma_start.
"""

import sys

for _p in ("/opt/trn_rl_repo", "/opt/trn_rl_repo/concourse"):
    if _p not in sys.path:
        sys.path.insert(0, _p)

import numpy as np

N_CORES = 8
BS, C, H, W = 16, 256, 64, 64
B_PER_CORE = BS // N_CORES  # 2
OH = OW = 13
CBLK = 2  # channel blocks of 128 per sample
NT = B_PER_CORE * CBLK  # 4 tiles of 128 partitions

_nc_cache = {}


def _build_nc(finalize=True):
    import concourse.bacc as bacc
    import concourse.mybir as mybir
    from concourse import tile
    from concourse.ap import AP as APc

    f32 = mybir.dt.float32
    bf16 = mybir.dt.bfloat16
    # Bacc (not bare Bass): its finalize() runs generate_event_semaphores,
    # which splits multi-sem sync waits that walrus cannot encode.
    nc = bacc.Bacc("TRN2", target_bir_lowering=False)
    x = nc.dram_tensor("x", [B_PER_CORE, C, H, W], f32, kind="ExternalInput")
    o = nc.dram_tensor(
        "out", [B_PER_CORE, 21 * C, OH, OW], bf16, kind="ExternalOutput"
    )

    def overlap(tap, start, dims):
        """Strided (possibly overlapping) free-dim view of a tile AP,
        starting at free-offset `start`.  Max 3 free dims (ISA limit)."""
        base = tap[:, start:]
        part = list(base.ap[0])
        return APc(
            tensor=base.tensor,
            offset=base.offset,
            ap=[part] + [[s, n] for (s, n) in dims],
        )

    with tile.TileContext(nc) as tc:
        with tc.tile_pool(name="sbuf", bufs=2) as pool:
            # Phase 1: enqueue ALL loads on the SP ring before any store
            # instruction can appear in its stream.  Half-tile loads
            # (32 rows, 8 KiB/partition descriptors) keep the per-line
            # size DMA-efficient while halving the row-max chunk that
            # hangs off the last load byte.
            xh = []
            for t in range(NT):
                b, cb = divmod(t, CBLK)
                cs = slice(cb * 128, (cb + 1) * 128)
                for ht in range(2):
                    xt = pool.tile([128, 2048], f32, tag="xh", bufs=2 * NT)
                    nc.sync.dma_start(
                        out=xt[:],
                        in_=x[b, cs, 32 * ht : 32 * (ht + 1)].rearrange(
                            "c h w -> c (h w)"
                        ),
                    )
                    xh.append(xt)

            # Phase 2: per-tile compute + stores.  Store dma_starts also
            # go on the SP ring: FIFO order guarantees they only drain
            # after all loads, and in copy-completion order.
            for t in range(NT):
                b, cb = divmod(t, CBLK)
                cs = slice(cb * 128, (cb + 1) * 128)
                r4 = pool.tile([128, 1024], bf16, tag="r4")
                for ht in range(2):
                    xt = xh[2 * t + ht]
                    bq = pool.tile([128, 1024], bf16, tag="bq", bufs=2)
                    xv = xt.rearrange("p (a t c) -> p a t c", t=2, c=W)
                    nc.vector.tensor_max(
                        out=bq.rearrange("p (a c) -> p a c", c=W),
                        in0=xv[:, :, 0, :],
                        in1=xv[:, :, 1, :],
                    )
                    bv = bq.rearrange("p (a t c) -> p a t c", t=2, c=W)
                    nc.vector.tensor_max(
                        out=r4[:, 512 * ht : 512 * (ht + 1)].rearrange(
                            "p (a c) -> p a c", c=W
                        ),
                        in0=bv[:, :, 0, :],
                        in1=bv[:, :, 1, :],
                    )
                # 4-col max: [16,64] -> P2 [16,16]
                c1 = pool.tile([128, 512], bf16, tag="c1")
                nc.vector.tensor_max(out=c1[:], in0=r4[:, 0::2], in1=r4[:, 1::2])
                p2 = pool.tile([128, 256], bf16, tag="p2")
                nc.vector.tensor_max(out=p2[:], in0=c1[:, 0::2], in1=c1[:, 1::2])

                # bufs=4: stores drain late (behind all loads in the ring),
                # so all four tiles' stages can be live at once.
                stage = pool.tile([128, 21 * OH * OW], bf16, tag="stage", bufs=4)

                lvl2_dst = o[
                    b, 1280 + cb * 2048 : 1280 + (cb + 1) * 2048
                ].rearrange("(c f) h w -> c (f h w)", f=16)
                # lvl2: 16 shifted 13x13 windows of P2 -> stage[845:3549]
                # (split over q: ISA mem patterns allow at most 3 free dims).
                # DMA instruction issue is ~0.65 us serial on Sync, so batch
                # stores coarsely: ONE lvl2 store per tile (two on the last
                # tile so drain pipelines with its copies).  On the last
                # tile, VectorE takes two of the four copies so the post-
                # load-critical chain is two copies deep, not four.
                last = t == NT - 1
                for q in range(4):
                    cdst = stage[:, (5 + 4 * q) * 169 : (9 + 4 * q) * 169]
                    csrc = overlap(p2, q * 16, [(1, 4), (16, 13), (1, 13)])
                    if last and q % 2 == 1:
                        nc.vector.tensor_copy(out=cdst, in_=csrc)
                    else:
                        nc.scalar.copy(out=cdst, in_=csrc)
                    if last and q == 1:
                        nc.sync.dma_start(
                            out=lvl2_dst[:, : 8 * 169],
                            in_=stage[:, 5 * 169 : 13 * 169],
                        )
                if last:
                    nc.sync.dma_start(
                        out=lvl2_dst[:, 8 * 169 :],
                        in_=stage[:, 13 * 169 : 21 * 169],
                    )
                else:
                    nc.sync.dma_start(
                        out=lvl2_dst[:],
                        in_=stage[:, 5 * 169 : 21 * 169],
                    )
                # P1 = 2x2 stride-1 max of P2 -> [15,15]
                t1 = pool.tile([128, 240], bf16, tag="t1")
                p2m = p2.rearrange("p (h w) -> p h w", w=16)
                nc.vector.tensor_max(
                    out=t1.rearrange("p (h w) -> p h w", w=15),
                    in0=p2m[:, :, 0:15],
                    in1=p2m[:, :, 1:16],
                )
                p1 = pool.tile([128, 225], bf16, tag="p1")
                nc.vector.tensor_max(out=p1[:], in0=t1[:, 0:225], in1=t1[:, 15:240])
                # lvl1: 4 shifted 13x13 windows of P1 (stride 2), gathered on
                # GpSimd (ScalarE is busy with the lvl2 copies).
                for q in range(2):
                    nc.gpsimd.tensor_copy(
                        out=stage[:, (1 + 2 * q) * 169 : (3 + 2 * q) * 169],
                        in_=overlap(p1, q * 30, [(2, 2), (15, 13), (1, 13)]),
                    )
                # P0 = 4x4 stride-1 max of P2 = 2x2 stride-2 max of P1
                t2 = pool.tile([128, 195], bf16, tag="t2")
                p1m = p1.rearrange("p (h w) -> p h w", w=15)
                nc.vector.tensor_max(
                    out=t2.rearrange("p (h w) -> p h w", w=13),
                    in0=p1m[:, :, 0:13],
                    in1=p1m[:, :, 2:15],
                )
                nc.vector.tensor_max(
                    out=stage[:, 0:169], in0=t2[:, 0:169], in1=t2[:, 26:195]
                )
                nc.sync.dma_start(
                    out=o[b, 256 + cb * 512 : 256 + (cb + 1) * 512].rearrange(
                        "(c f) h w -> c (f h w)", f=4
                    ),
                    in_=stage[:, 169 : 5 * 169],
                )
                nc.sync.dma_start(
                    out=o[b, cs].rearrange("c h w -> c (h w)"),
                    in_=stage[:, 0:169],
                )
    if finalize:
        nc.finalize()
    return nc


def get_nc():
    if "nc" not in _nc_cache:
        _nc_cache["nc"] = _build_nc()
    return _nc_cache["nc"]


def kernel(x: np.ndarray, _trace: bool = False):
    from concourse.bass_utils import run_bass_kernel_spmd

    x = np.ascontiguousarray(np.asarray(x), dtype=np.float32)
    assert x.shape == (BS, C, H, W), x.shape
    nc = get_nc()
    in_maps = [
        {"x": x[c * B_PER_CORE : (c + 1) * B_PER_CORE]} for c in range(N_CORES)
    ]
    res = run_bass_kernel_spmd(
        nc, in_maps, core_ids=list(range(N_CORES)), trace=_trace
    )
    out = np.concatenate(
        [np.asarray(r["out"]).astype(np.float32) for r in res.results], axis=0
    )
    if _trace:
        return out, res
    return out


# revision 9
# speedup vs baseline: 1.0943x; 1.0943x over previous
"""SPP (spatial pyramid pooling) kernel for Trainium2, 8 NeuronCores.

Input  x  : [16, 256, 64, 64] f32
Output    : [16, 5376, 13, 13] f32

Math: windows are 16x16 at stride 4 -> 13x13 window grid. Levels use
sub-cells of 16/8/4 pixels, all aligned to multiples of 4, so everything
reduces to the non-overlapping 4x4 block-max P2 [16,16] per (b,c) image:
  lvl2 plane (q,r) = P2[q+i, r+j]              (16 planes of 13x13)
  P1 = 2x2 stride-1 max of P2 -> [15,15];  lvl1 plane (q,r) = P1[2q+i, 2r+j]
  P0 = 4x4 stride-1 max of P2 -> [13,13];  lvl0 plane    = P0
Output channel order: [lvl0: c][lvl1: c*4+q*2+r][lvl2: c*16+q*4+r].

Precision: the input is cast f32->bf16 during the load DMA (SWDGE CCE
cast).  Max commutes exactly with monotone round-to-nearest, so every
output equals the bf16 rounding of the exact f32 max: rel err <= 2^-9,
vs the 2e-2 harness gate.  Output DRAM tensor is bf16; host upcasts.

Why bf16 loads: (1) DVE reads 16-bit at 2x element rate, so the first
row-max pass (the dominant VectorE cost: it reads every input element)
halves; (2) SBUF AXI-port bytes for loads halve (the measured ~430 GB/s
steady-state ceiling sits at the 435 GB/s SBUF DMA fabric spec).

Schedule: ALL DMA (loads and stores) goes on the single SWDGE ring,
which drains strictly FIFO.  The 8 half-tile loads are enqueued
up-front so the load stream runs with zero store competition; the bf16
stores sit behind them in the ring and drain during the last tile's
gather-copy tail.  GpSimd issues DMA only (its COPY/compute ops
measurably poison DVE throughput ~2x, so lvl1 gathers live on VectorE
and lvl2 gathers on ScalarE; the last tile splits lvl2 gathers between
both to shorten the final chain).  Store DMAs are coarse (1 lvl2 store
per tile; 2 on the last tile) because DMA instruction issue is ~0.65 us
serial per instruction.
"""

import sys

for _p in ("/opt/trn_rl_repo", "/opt/trn_rl_repo/concourse"):
    if _p not in sys.path:
        sys.path.insert(0, _p)

import numpy as np

N_CORES = 8
BS, C, H, W = 16, 256, 64, 64
B_PER_CORE = BS // N_CORES  # 2
OH = OW = 13
CBLK = 2  # channel blocks of 128 per sample
NT = B_PER_CORE * CBLK  # 4 tiles of 128 partitions

_nc_cache = {}


def _build_nc(finalize=True):
    import concourse.bacc as bacc
    import concourse.mybir as mybir
    from concourse import tile
    from concourse.ap import AP as APc

    f32 = mybir.dt.float32
    bf16 = mybir.dt.bfloat16
    # Bacc (not bare Bass): its finalize() runs generate_event_semaphores,
    # which splits multi-sem sync waits that walrus cannot encode.
    nc = bacc.Bacc("TRN2", target_bir_lowering=False)
    x = nc.dram_tensor("x", [B_PER_CORE, C, H, W], f32, kind="ExternalInput")
    o = nc.dram_tensor(
        "out", [B_PER_CORE, 21 * C, OH, OW], bf16, kind="ExternalOutput"
    )

    def overlap(tap, start, dims):
        """Strided (possibly overlapping) free-dim view of a tile AP,
        starting at free-offset `start`.  Max 3 free dims (ISA limit)."""
        base = tap[:, start:]
        part = list(base.ap[0])
        return APc(
            tensor=base.tensor,
            offset=base.offset,
            ap=[part] + [[s, n] for (s, n) in dims],
        )

    with tile.TileContext(nc) as tc:
        with tc.tile_pool(name="sbuf", bufs=2) as pool:
            # Phase 1: enqueue ALL loads (f32 -> bf16 cast during DMA) on
            # the SWDGE ring before any store instruction appears in the
            # gpsimd stream.
            xh = []
            for t in range(NT):
                b, cb = divmod(t, CBLK)
                cs = slice(cb * 128, (cb + 1) * 128)
                for ht in range(2):
                    xt = pool.tile([128, 2048], bf16, tag="xh", bufs=2 * NT)
                    nc.gpsimd.dma_start(
                        out=xt[:],
                        in_=x[b, cs, 32 * ht : 32 * (ht + 1)].rearrange(
                            "c h w -> c (h w)"
                        ),
                    )
                    xh.append(xt)

            # Phase 2: per-tile compute + stores (also on the SWDGE ring:
            # FIFO order = after all loads, in copy-completion order).
            for t in range(NT):
                b, cb = divmod(t, CBLK)
                cs = slice(cb * 128, (cb + 1) * 128)
                last = t == NT - 1
                r4 = pool.tile([128, 1024], bf16, tag="r4")
                for ht in range(2):
                    xt = xh[2 * t + ht]
                    bq = pool.tile([128, 1024], bf16, tag="bq", bufs=2)
                    xv = xt.rearrange("p (a t c) -> p a t c", t=2, c=W)
                    nc.vector.tensor_max(
                        out=bq.rearrange("p (a c) -> p a c", c=W),
                        in0=xv[:, :, 0, :],
                        in1=xv[:, :, 1, :],
                    )
                    bv = bq.rearrange("p (a t c) -> p a t c", t=2, c=W)
                    nc.vector.tensor_max(
                        out=r4[:, 512 * ht : 512 * (ht + 1)].rearrange(
                            "p (a c) -> p a c", c=W
                        ),
                        in0=bv[:, :, 0, :],
                        in1=bv[:, :, 1, :],
                    )
                # 4-col max: [16,64] -> P2 [16,16]
                c1 = pool.tile([128, 512], bf16, tag="c1")
                nc.vector.tensor_max(out=c1[:], in0=r4[:, 0::2], in1=r4[:, 1::2])
                p2 = pool.tile([128, 256], bf16, tag="p2")
                nc.vector.tensor_max(out=p2[:], in0=c1[:, 0::2], in1=c1[:, 1::2])

                stage = pool.tile([128, 21 * OH * OW], bf16, tag="stage", bufs=4)

                lvl2_dst = o[
                    b, 1280 + cb * 2048 : 1280 + (cb + 1) * 2048
                ].rearrange("(c f) h w -> c (f h w)", f=16)
                # lvl2: 16 shifted 13x13 windows of P2 -> stage[845:3549]
                # (split over q: ISA mem patterns allow at most 3 free dims).
                for q in range(4):
                    cdst = stage[:, (5 + 4 * q) * 169 : (9 + 4 * q) * 169]
                    csrc = overlap(p2, q * 16, [(1, 4), (16, 13), (1, 13)])
                    if last and q % 2 == 1:
                        nc.vector.tensor_copy(out=cdst, in_=csrc)
                    else:
                        nc.scalar.copy(out=cdst, in_=csrc)
                    if last and q == 1:
                        nc.gpsimd.dma_start(
                            out=lvl2_dst[:, : 8 * 169],
                            in_=stage[:, 5 * 169 : 13 * 169],
                        )
                if last:
                    nc.gpsimd.dma_start(
                        out=lvl2_dst[:, 8 * 169 :],
                        in_=stage[:, 13 * 169 : 21 * 169],
                    )
                else:
                    nc.gpsimd.dma_start(
                        out=lvl2_dst[:],
                        in_=stage[:, 5 * 169 : 21 * 169],
                    )
                # P1 = 2x2 stride-1 max of P2 -> [15,15]
                t1 = pool.tile([128, 240], bf16, tag="t1")
                p2m = p2.rearrange("p (h w) -> p h w", w=16)
                nc.vector.tensor_max(
                    out=t1.rearrange("p (h w) -> p h w", w=15),
                    in0=p2m[:, :, 0:15],
                    in1=p2m[:, :, 1:16],
                )
                p1 = pool.tile([128, 225], bf16, tag="p1")
                nc.vector.tensor_max(out=p1[:], in0=t1[:, 0:225], in1=t1[:, 15:240])
                # lvl1: 4 shifted 13x13 windows of P1 (stride 2) on VectorE.
                for q in range(2):
                    nc.vector.tensor_copy(
                        out=stage[:, (1 + 2 * q) * 169 : (3 + 2 * q) * 169],
                        in_=overlap(p1, q * 30, [(2, 2), (15, 13), (1, 13)]),
                    )
                # P0 = 4x4 stride-1 max of P2 = 2x2 stride-2 max of P1
                t2 = pool.tile([128, 195], bf16, tag="t2")
                p1m = p1.rearrange("p (h w) -> p h w", w=15)
                nc.vector.tensor_max(
                    out=t2.rearrange("p (h w) -> p h w", w=13),
                    in0=p1m[:, :, 0:13],
                    in1=p1m[:, :, 2:15],
                )
                nc.vector.tensor_max(
                    out=stage[:, 0:169], in0=t2[:, 0:169], in1=t2[:, 26:195]
                )
                nc.gpsimd.dma_start(
                    out=o[b, 256 + cb * 512 : 256 + (cb + 1) * 512].rearrange(
                        "(c f) h w -> c (f h w)", f=4
                    ),
                    in_=stage[:, 169 : 5 * 169],
                )
                nc.gpsimd.dma_start(
                    out=o[b, cs].rearrange("c h w -> c (h w)"),
                    in_=stage[:, 0:169],
                )
    if finalize:
        nc.finalize()
    return nc


def get_nc():
    if "nc" not in _nc_cache:
        _nc_cache["nc"] = _build_nc()
    return _nc_cache["nc"]


def kernel(x: np.ndarray, _trace: bool = False):
    from concourse.bass_utils import run_bass_kernel_spmd

    x = np.ascontiguousarray(np.asarray(x), dtype=np.float32)
    assert x.shape == (BS, C, H, W), x.shape
    nc = get_nc()
    in_maps = [
        {"x": x[c * B_PER_CORE : (c + 1) * B_PER_CORE]} for c in range(N_CORES)
    ]
    res = run_bass_kernel_spmd(
        nc, in_maps, core_ids=list(range(N_CORES)), trace=_trace
    )
    out = np.concatenate(
        [np.asarray(r["out"]).astype(np.float32) for r in res.results], axis=0
    )
    if _trace:
        return out, res
    return out
